# revision 37
# baseline (speedup 1.0000x reference)
"""YOLO-v2 loss kernel for Trainium2 (8 NeuronCores, data-parallel over batch).

Decomposition (same partial sums as the validated baseline):
  stack cols = [dense_sigmoid_sq_A, pos_mse, pos_psq, npos, cls_num, txy, twh,
                dense_sigmoid_sq_B]
summed over 128 partitions by one PE matmul, combined on the host.

The GT matching (anchors IoU, argmax, cell assignment, last-writer-wins and
ignore-overwrite masks) depends only on gboxes/labels (4KB of input), so it is
precomputed on the host into per-slot constants + gather offsets, exactly like
the baseline already precomputed its per-slot channel offsets.  Everything
touching pyolos (the 147MB tensor) stays on-device:
  - conf planes (16x5x676 per core) DMA'd + sigmoid^2-accumulate split between
    the ACT engine (3-pass exp/ln, cols 0:LD) and the DVE (recip path, rest)
  - 90 channel values per slot fetched with one indirect DMA
  - decode/IoU/loss math on DVE+Pool in parallel strands

Key trick: columns are gathered in the order
  [conf, tx, ty, tx, ty, tw, th, tw, th, lbl_ch, cls0..79]
with tx/ty duplicated so one Exp(-x) activation plus a DVE reciprocal yields
both sigmoid(t) and sigmoid(-t) = 1 - sigmoid(t); the prb / -plt box corners
then come out of a single add against host-folded constants, and the IoU
min/max pair collapses to one tensor_tensor min.
"""

import numpy as np

from concourse import bass, mybir
from concourse.bass_utils import run_bass_kernel_spmd
from concourse.tile import TileContext

F32 = mybir.dt.float32
I32 = mybir.dt.int32
AF = mybir.ActivationFunctionType
OP = mybir.AluOpType
AX = mybir.AxisListType

NC = 8                 # cores
B = 128                # batch
BL = B // NC           # images per core (16)
NGT = 8                # GTs per image
S = BL * NGT           # slots per core (128)
GRID = 26
HW = GRID * GRID       # 676
IMG = 425 * HW         # elements per image
EPS = 1e-7
NCOL = 90              # gathered columns per slot
NCONST = 24            # f32 const columns
NMETA = NCOL + NCONST  # i32 meta columns (consts bitcast)
LD = 376               # dense-conf columns handled by ACT (rest on DVE)
ANC = np.array([[0.05, 0.07], [0.12, 0.15], [0.25, 0.30],
                [0.45, 0.50], [0.80, 0.85]], np.float32)

# const f32 column offsets (within the 24-col block)
C_C4 = 0      # [4]  [cr0, cr1, -1-cr0, -1-cr1]
C_AH4 = 4     # [4]  [aw, ah, aw, ah] * 13
C_GB4 = 8     # [4]  [r, b, -l, -t] * 26
C_AGE = 12    # [1]  gt area*676 + eps*676
C_LW = 13     # [1]  last-writer mask
C_LWE = 14    # [1]  lw * weff
C_ZC = 15     # [2]  1 - txy target
C_TWT = 17    # [2]  twh target
C_ONE = 19    # [1]  1.0


def _split_multiwaits(nc: bass.Bass, k: int = 1) -> None:
    """This walrus build rejects instructions with >~2 sync waits; hoist
    extra waits onto preceding same-engine NoOps."""
    for fn in nc.m.functions:
        for bb in fn.blocks:
            out = []
            for inst in bb.instructions:
                si = inst.sync_info
                waits = list(si.on_wait) if si is not None and si.on_wait else []
                if len(waits) > k:
                    for i, w in enumerate(waits[:-k]):
                        out.append(mybir.InstNoOp(
                            name=f"{inst.name}-wsplit{i}",
                            engine=inst.engine,
                            bass_nofuse=True,
                            sync_info=mybir.SyncInfo(on_wait=[w],
                                                     on_update=[]),
                        ))
                    inst.sync_info = mybir.SyncInfo(
                        on_wait=waits[-k:], on_update=list(si.on_update))
                out.append(inst)
            bb.instructions = out


def _host_match(gbx: np.ndarray, lbl: np.ndarray):
    """Vectorized fmatch4yolov2 mirror (f32, matches the jax reference).
    gbx [B,8,4] ltrb, lbl [B,8] 1-based.  Returns per-slot meta arrays."""
    gbx = gbx.astype(np.float32)
    cxy = (gbx[..., :2] + gbx[..., 2:]) * np.float32(0.5)
    wh = gbx[..., 2:] - gbx[..., :2]
    inter = np.minimum(wh[..., None, :], ANC[None, None]).prod(-1)
    areag = wh.prod(-1)
    iou2 = inter / (areag[..., None] + (ANC[:, 0] * ANC[:, 1])[None, None]
                    - inter + np.float32(EPS))
    mign = iou2 > 0.5                                   # [B,8,5]
    idxm = iou2.argmax(-1)                              # [B,8]
    colrow = (cxy * np.float32(GRID)).astype(np.int32)  # trunc == floor here
    crf = colrow.astype(np.float32)
    txy = cxy * np.float32(GRID) - crf
    twh = np.log(wh / ANC[idxm])
    weight = np.float32(2.0) - areag
    cell = colrow[..., 1] * GRID + colrow[..., 0]       # [B,8] int
    key = cell * 5 + idxm

    # upper-triangular (j > i) collision masks
    jgt = np.triu(np.ones((NGT, NGT), bool), 1)[None]   # [1,i,j]
    same_key = key[:, :, None] == key[:, None, :]       # [b,i,j]
    lastw = ~np.logical_and(same_key, jgt).any(-1)      # [B,8]
    same_cell = cell[:, :, None] == cell[:, None, :]
    # mji[b,i,j] = mign[b, j, idxm[b, i]]
    mji = np.take_along_axis(
        mign.transpose(0, 2, 1),                        # [b, a, j]
        idxm[:, :, None], axis=1)                       # [b, i, j]
    ignov = np.logical_and(np.logical_and(same_cell, jgt), mji).any(-1)
    weff = np.where(ignov, np.float32(-1.0), weight)
    return dict(idxm=idxm, crf=crf, txy=txy, twh=twh, cell=cell,
                lastw=lastw.astype(np.float32), weff=weff.astype(np.float32),
                areag=areag, gbx=gbx)


def _make_meta(m: dict, sl: slice) -> np.ndarray:
    """Pack per-slot gather offsets + f32 consts for one core -> i32 [S,NMETA]."""
    idxm = m["idxm"][sl].reshape(S)
    cell = m["cell"][sl].reshape(S)
    crf = m["crf"][sl].reshape(S, 2)
    txy = m["txy"][sl].reshape(S, 2)
    twh = m["twh"][sl].reshape(S, 2)
    lastw = m["lastw"][sl].reshape(S)
    weff = m["weff"][sl].reshape(S)
    areag = m["areag"][sl].reshape(S)
    gbx = m["gbx"][sl].reshape(S, 4)
    lblch = m["lblch"][sl].reshape(S)

    img = np.arange(S) // NGT
    base = img * IMG + idxm * HW + cell                 # [S]
    # channel planes: conf=0, cls k -> 1+k, tx..th -> 81..84
    ch = np.concatenate([
        np.array([0, 81, 82, 81, 82, 83, 84, 83, 84]),
        np.zeros(1, np.int64),                          # placeholder for lbl
        np.arange(1, 81),
    ])
    offs = base[:, None] + ch[None, :] * (5 * HW)
    offs[:, 9] = base + lblch * (5 * HW)
    meta = np.zeros((S, NMETA), np.int32)
    meta[:, :NCOL] = offs.astype(np.int32)

    ct = np.zeros((S, NCONST), np.float32)
    anc = ANC[idxm]                                     # [S,2]
    ct[:, C_C4 + 0:C_C4 + 2] = crf
    ct[:, C_C4 + 2:C_C4 + 4] = -1.0 - crf
    ct[:, C_AH4 + 0:C_AH4 + 2] = anc * (GRID / 2.0)
    ct[:, C_AH4 + 2:C_AH4 + 4] = anc * (GRID / 2.0)
    ct[:, C_GB4 + 0] = gbx[:, 2] * GRID
    ct[:, C_GB4 + 1] = gbx[:, 3] * GRID
    ct[:, C_GB4 + 2] = -gbx[:, 0] * GRID
    ct[:, C_GB4 + 3] = -gbx[:, 1] * GRID
    ct[:, C_AGE] = areag * (HW * 1.0) + EPS * HW
    ct[:, C_LW] = lastw
    ct[:, C_LWE] = lastw * weff
    ct[:, C_ZC:C_ZC + 2] = 1.0 - txy
    ct[:, C_TWT:C_TWT + 2] = twh
    ct[:, C_ONE] = 1.0
    meta[:, NCOL:] = ct.view(np.int32)
    return meta


def build_bass() -> bass.Bass:
    nc = bass.Bass()
    py = nc.declare_dram_parameter("pyolos", [BL, 425, HW], F32, isOutput=False)
    mtd = nc.declare_dram_parameter("meta", [S, NMETA], I32, isOutput=False)
    out = nc.declare_dram_parameter("out", [1, 8], F32, isOutput=True)
    py_flat = py[:, :, :].rearrange("a b c -> (a b c)")

    with TileContext(nc) as tc:
        with (
            tc.tile_pool(name="sb", bufs=1) as sb,
            tc.tile_pool(name="ps", bufs=1, space="PSUM") as ps,
        ):
            mt = sb.tile([S, NMETA], I32, name="mt")
            conf = sb.tile([BL * 5, HW], F32, name="conf")
            sq = sb.tile([BL * 5, HW], F32, name="sq")
            pf = sb.tile([S, NCOL], F32, name="pf")
            u9 = sb.tile([S, 9], F32, name="u9")
            sp80 = sb.tile([S, 80], F32, name="sp80")
            stack = sb.tile([S, 8], F32, name="stack")

            def ctf(c0, c1):
                return mt[:, NCOL + c0:NCOL + c1].bitcast(F32)

            def tt(shape, tag):
                return sb.tile(shape, F32, name=tag)

            # ---- DMAs: meta on SP; conf halves split across both HWDGE
            # queues so pass 1 can start earlier ----
            CH = 338
            nc.sync.dma_start(out=mt[:], in_=mtd[:, :])
            nc.scalar.dma_start(out=conf[:, 0:CH], in_=py[:, 0:5, 0:CH])
            nc.sync.dma_start(out=conf[:, CH:HW], in_=py[:, 0:5, CH:HW])

            # ---- indirect gathers on gpsimd (critical 10 cols first) ----
            in1 = py_flat.rearrange("(a b) -> a b", b=1)
            nc.gpsimd.indirect_dma_start(
                out=pf[:, 0:10], out_offset=None, in_=in1,
                in_offset=bass.IndirectOffsetOnAxis(ap=mt[:, 0:10], axis=0))
            nc.gpsimd.indirect_dma_start(
                out=pf[:, 10:NCOL], out_offset=None, in_=in1,
                in_offset=bass.IndirectOffsetOnAxis(ap=mt[:, 10:NCOL], axis=0))

            # ---- DVE early (independent of gathers): zero the stack ----
            nc.vector.memset(stack[:], 0.0)

            # ---- ACT queue.  Dense conf background
            # sigmoid(x)^2 = exp(-2*ln(1+e^-x)) in 3 passes, column-split so
            # the per-slot activations interleave at pf/cls arrival. ----
            nc.scalar.activation(sq[:, 0:CH], conf[:, 0:CH], AF.Exp,
                                 scale=-1.0)
            nc.scalar.activation(sq[:, CH:HW], conf[:, CH:HW], AF.Exp,
                                 scale=-1.0)
            nc.scalar.activation(sq[:, 0:CH], sq[:, 0:CH], AF.Ln, bias=1.0)
            nc.scalar.activation(u9[:, 0:3], pf[:, 0:3], AF.Exp, scale=-1.0)
            nc.scalar.activation(u9[:, 3:9], pf[:, 3:9], AF.Exp)
            spl = tt([S, 2], "spl")
            nc.scalar.activation(spl[:], u9[:, 1:3], AF.Ln, bias=1.0)
            nc.scalar.activation(sq[:, CH:HW], sq[:, CH:HW], AF.Ln, bias=1.0)
            nc.scalar.activation(sq[:, 0:CH], sq[:, 0:CH], AF.Exp, scale=-2.0,
                                 accum_out=stack[0:BL * 5, 0:1])
            spsum = tt([S, 1], "spsum")
            nc.scalar.activation(sp80[:], pf[:, 10:NCOL], AF.Exp)
            nc.scalar.activation(sp80[:], sp80[:], AF.Ln, bias=1.0,
                                 accum_out=spsum[:])
            nc.scalar.activation(sq[:, CH:HW], sq[:, CH:HW], AF.Exp,
                                 scale=-2.0,
                                 accum_out=stack[0:BL * 5, 7:8])

            # ---- DVE pre-chain (fills the wait for u3/e6): bce-xy / mse-wh
            # elementwise pieces that only need the gathered columns ----
            t2 = tt([S, 2], "t2")
            nc.vector.tensor_tensor(out=t2[:], in0=pf[:, 1:3],
                                    in1=ctf(C_ZC, C_ZC + 2), op=OP.mult)
            dwh = tt([S, 2], "dwh")
            nc.vector.tensor_tensor(out=dwh[:], in0=pf[:, 5:7],
                                    in1=ctf(C_TWT, C_TWT + 2), op=OP.subtract)
            dsq = tt([S, 2], "dsq")
            nc.vector.tensor_tensor(out=dsq[:], in0=dwh[:], in1=dwh[:],
                                    op=OP.mult)
            msewh = tt([S, 1], "msewh")
            nc.vector.tensor_tensor(out=msewh[:], in0=dsq[:, 0:1],
                                    in1=dsq[:, 1:2], op=OP.add)

            # ---- Pool strand: pwh/area pieces feeding the DVE chain ----
            w4 = tt([S, 4], "w4")
            nc.gpsimd.tensor_tensor(out=w4[:], in0=u9[:, 5:9],
                                    in1=ctf(C_AH4, C_AH4 + 4), op=OP.mult)
            w4c = tt([S, 4], "w4c")
            nc.gpsimd.tensor_tensor(out=w4c[:], in0=w4[:],
                                    in1=ctf(C_C4, C_C4 + 4), op=OP.add)
            pa = tt([S, 1], "pa")
            nc.gpsimd.tensor_tensor(out=pa[:], in0=w4[:, 0:1], in1=w4[:, 1:2],
                                    op=OP.mult)
            pa_ag = tt([S, 1], "pa_ag")
            nc.gpsimd.tensor_scalar(pa_ag[:], pa[:], 4.0,
                                    ctf(C_AGE, C_AGE + 1), OP.mult, OP.add)

            # ---- DVE decode/IoU chain ----
            v5 = tt([S, 5], "v5")
            nc.vector.tensor_scalar(v5[:], u9[:, 0:5], 1.0, None, OP.add)
            sig5 = tt([S, 5], "sig5")
            nc.vector.reciprocal(sig5[:], v5[:])
            q4 = tt([S, 4], "q4")
            nc.vector.tensor_tensor(out=q4[:], in0=sig5[:, 1:5], in1=w4c[:],
                                    op=OP.add)
            q4m = tt([S, 4], "q4m")
            nc.vector.tensor_tensor(out=q4m[:], in0=q4[:],
                                    in1=ctf(C_GB4, C_GB4 + 4), op=OP.min)
            s2 = tt([S, 2], "s2")
            nc.vector.tensor_tensor(out=s2[:], in0=q4m[:, 0:2],
                                    in1=q4m[:, 2:4], op=OP.add)
            iwh = tt([S, 2], "iwh")
            nc.vector.tensor_scalar(iwh[:], s2[:], 0.0, None, OP.max)
            inter = tt([S, 1], "inter")
            nc.vector.tensor_tensor(out=inter[:], in0=iwh[:, 0:1],
                                    in1=iwh[:, 1:2], op=OP.mult)
            den = tt([S, 1], "den")
            nc.vector.scalar_tensor_tensor(out=den[:], in0=inter[:],
                                           scalar=-1.0, in1=pa_ag[:],
                                           op0=OP.mult, op1=OP.add)
            deni = tt([S, 1], "deni")
            nc.vector.reciprocal(deni[:], den[:])
            gconf = tt([S, 1], "gconf")
            nc.vector.tensor_tensor(out=gconf[:], in0=inter[:], in1=deni[:],
                                    op=OP.mult)
            # mp = (gconf > 0) * lastw, written straight into the npos col
            nc.vector.tensor_scalar(stack[:, 3:4], gconf[:], 0.0,
                                    ctf(C_LW, C_LW + 1), OP.is_gt, OP.mult)
            mpw = tt([S, 1], "mpw")
            nc.vector.tensor_scalar(mpw[:], gconf[:], 0.0,
                                    ctf(C_LWE, C_LWE + 1), OP.is_gt, OP.mult)
            dconf = tt([S, 1], "dconf")
            nc.vector.tensor_tensor(out=dconf[:], in0=sig5[:, 0:1],
                                    in1=gconf[:], op=OP.subtract)
            nc.vector.scalar_tensor_tensor(out=stack[:, 1:2], in0=dconf[:],
                                           scalar=stack[:, 3:4], in1=dconf[:],
                                           op0=OP.mult, op1=OP.mult)
            nc.vector.scalar_tensor_tensor(out=stack[:, 2:3], in0=sig5[:, 0:1],
                                           scalar=stack[:, 3:4],
                                           in1=sig5[:, 0:1],
                                           op0=OP.mult, op1=OP.mult)

            # ---- Pool strand: remaining stack columns ----
            nc.gpsimd.tensor_scalar(stack[:, 6:7], msewh[:], mpw[:], None,
                                    OP.mult)
            sptxy = tt([S, 2], "sptxy")
            nc.gpsimd.tensor_tensor(out=sptxy[:], in0=spl[:], in1=t2[:],
                                    op=OP.add)
            bcexy = tt([S, 1], "bcexy")
            nc.gpsimd.tensor_tensor(out=bcexy[:], in0=sptxy[:, 0:1],
                                    in1=sptxy[:, 1:2], op=OP.add)
            nc.gpsimd.tensor_scalar(stack[:, 5:6], bcexy[:], mpw[:], None,
                                    OP.mult)

            # ---- DVE tail: cls column via mp*spsum - mp*xlab ----
            xm = tt([S, 1], "xm")
            nc.vector.tensor_tensor(out=xm[:], in0=pf[:, 9:10],
                                    in1=stack[:, 3:4], op=OP.mult)
            nc.vector.scalar_tensor_tensor(out=stack[:, 4:5], in0=spsum[:],
                                           scalar=stack[:, 3:4], in1=xm[:],
                                           op0=OP.mult, op1=OP.subtract)

            # ---- final reduce over partitions + output ----
            red = ps.tile([1, 8], F32, name="red")
            nc.tensor.matmul(out=red[:], lhsT=ctf(C_ONE, C_ONE + 1),
                             rhs=stack[:], start=True, stop=True)
            osb = sb.tile([1, 8], F32, name="osb")
            nc.vector.tensor_copy(osb[:], red[:])
            nc.sync.dma_start(out=out[:, :], in_=osb[:])
    _split_multiwaits(nc, k=1)
    return nc


_NC_CACHE = None
LAST_RESULTS = None


def _get_nc():
    global _NC_CACHE
    if _NC_CACHE is None:
        _NC_CACHE = build_bass()
    return _NC_CACHE


def run(pyolos, gboxes_ltrb, labels, trace=False, **spmd_kwargs):
    global LAST_RESULTS
    nc = _get_nc()
    py = np.ascontiguousarray(
        np.asarray(pyolos, np.float32).reshape(B, 425, HW))
    gbx = np.asarray(gboxes_ltrb, np.float32).reshape(B, NGT, 4)
    lbl = np.asarray(labels).reshape(B, NGT).astype(np.int64)
    m = _host_match(gbx, lbl)
    m["lblch"] = lbl  # class channel plane index is exactly the 1-based label
    in_maps = []
    for c in range(NC):
        sl = slice(c * BL, (c + 1) * BL)
        in_maps.append({
            "pyolos": py[sl],
            "meta": _make_meta(m, sl),
        })
    res = run_bass_kernel_spmd(nc, in_maps, list(range(NC)), trace=trace,
                               **spmd_kwargs)
    LAST_RESULTS = res
    outs = np.stack([r["out"][0] for r in res.results]).astype(np.float64)
    t = outs.sum(0)
    dense_sq = t[0] + t[7]
    pos_mse, pos_psq, npos, cls_num, txy_s, twh_s = t[1:7]
    loss = (5.0 * pos_mse / B
            + (dense_sq - pos_psq) / B
            + cls_num / max(npos, 1.0)
            + txy_s / B
            + twh_s / B)
    return np.float32(loss)


def kernel(pyolos, gboxes_ltrb, labels):
    return run(pyolos, gboxes_ltrb, labels)


# revision 38
# speedup vs baseline: 1.0088x; 1.0088x over previous
"""YOLO-v2 loss kernel for Trainium2 (8 NeuronCores, data-parallel over batch).

Decomposition (same partial sums as the validated baseline):
  stack cols = [dense_sigmoid_sq_A, pos_mse, pos_psq, npos, cls_num, txy, twh,
                dense_sigmoid_sq_B]
summed over 128 partitions by one PE matmul, combined on the host.

The GT matching (anchors IoU, argmax, cell assignment, last-writer-wins and
ignore-overwrite masks) depends only on gboxes/labels (4KB of input), so it is
precomputed on the host into per-slot constants + gather offsets, exactly like
the baseline already precomputed its per-slot channel offsets.  Everything
touching pyolos (the 147MB tensor) stays on-device:
  - conf planes (16x5x676 per core) DMA'd + sigmoid^2-accumulate split between
    the ACT engine (3-pass exp/ln, cols 0:LD) and the DVE (recip path, rest)
  - 90 channel values per slot fetched with one indirect DMA
  - decode/IoU/loss math on DVE+Pool in parallel strands

Key trick: columns are gathered in the order
  [conf, tx, ty, tx, ty, tw, th, tw, th, lbl_ch, cls0..79]
with tx/ty duplicated so one Exp(-x) activation plus a DVE reciprocal yields
both sigmoid(t) and sigmoid(-t) = 1 - sigmoid(t); the prb / -plt box corners
then come out of a single add against host-folded constants, and the IoU
min/max pair collapses to one tensor_tensor min.
"""

import os

os.environ.setdefault("TILE_SCHEDULER", "asap")

import numpy as np

from concourse import bass, mybir
from concourse.bass_utils import run_bass_kernel_spmd
from concourse.tile import TileContext

F32 = mybir.dt.float32
I32 = mybir.dt.int32
AF = mybir.ActivationFunctionType
OP = mybir.AluOpType
AX = mybir.AxisListType

NC = 8                 # cores
B = 128                # batch
BL = B // NC           # images per core (16)
NGT = 8                # GTs per image
S = BL * NGT           # slots per core (128)
GRID = 26
HW = GRID * GRID       # 676
IMG = 425 * HW         # elements per image
EPS = 1e-7
NCOL = 90              # gathered columns per slot
NCONST = 24            # f32 const columns
NMETA = NCOL + NCONST  # i32 meta columns (consts bitcast)
LD = 376               # dense-conf columns handled by ACT (rest on DVE)
ANC = np.array([[0.05, 0.07], [0.12, 0.15], [0.25, 0.30],
                [0.45, 0.50], [0.80, 0.85]], np.float32)

# const f32 column offsets (within the 24-col block)
C_C4 = 0      # [4]  [cr0, cr1, -1-cr0, -1-cr1]
C_AH4 = 4     # [4]  [aw, ah, aw, ah] * 13
C_GB4 = 8     # [4]  [r, b, -l, -t] * 26
C_AGE = 12    # [1]  gt area*676 + eps*676
C_LW = 13     # [1]  last-writer mask
C_LWE = 14    # [1]  lw * weff
C_ZC = 15     # [2]  1 - txy target
C_TWT = 17    # [2]  twh target
C_ONE = 19    # [1]  1.0


def _split_multiwaits(nc: bass.Bass, k: int = 1) -> None:
    """This walrus build rejects instructions with >~2 sync waits; hoist
    extra waits onto preceding same-engine NoOps."""
    for fn in nc.m.functions:
        for bb in fn.blocks:
            out = []
            for inst in bb.instructions:
                si = inst.sync_info
                waits = list(si.on_wait) if si is not None and si.on_wait else []
                if len(waits) > k:
                    for i, w in enumerate(waits[:-k]):
                        out.append(mybir.InstNoOp(
                            name=f"{inst.name}-wsplit{i}",
                            engine=inst.engine,
                            bass_nofuse=True,
                            sync_info=mybir.SyncInfo(on_wait=[w],
                                                     on_update=[]),
                        ))
                    inst.sync_info = mybir.SyncInfo(
                        on_wait=waits[-k:], on_update=list(si.on_update))
                out.append(inst)
            bb.instructions = out


def _host_match(gbx: np.ndarray, lbl: np.ndarray):
    """Vectorized fmatch4yolov2 mirror (f32, matches the jax reference).
    gbx [B,8,4] ltrb, lbl [B,8] 1-based.  Returns per-slot meta arrays."""
    gbx = gbx.astype(np.float32)
    cxy = (gbx[..., :2] + gbx[..., 2:]) * np.float32(0.5)
    wh = gbx[..., 2:] - gbx[..., :2]
    inter = np.minimum(wh[..., None, :], ANC[None, None]).prod(-1)
    areag = wh.prod(-1)
    iou2 = inter / (areag[..., None] + (ANC[:, 0] * ANC[:, 1])[None, None]
                    - inter + np.float32(EPS))
    mign = iou2 > 0.5                                   # [B,8,5]
    idxm = iou2.argmax(-1)                              # [B,8]
    colrow = (cxy * np.float32(GRID)).astype(np.int32)  # trunc == floor here
    crf = colrow.astype(np.float32)
    txy = cxy * np.float32(GRID) - crf
    twh = np.log(wh / ANC[idxm])
    weight = np.float32(2.0) - areag
    cell = colrow[..., 1] * GRID + colrow[..., 0]       # [B,8] int
    key = cell * 5 + idxm

    # upper-triangular (j > i) collision masks
    jgt = np.triu(np.ones((NGT, NGT), bool), 1)[None]   # [1,i,j]
    same_key = key[:, :, None] == key[:, None, :]       # [b,i,j]
    lastw = ~np.logical_and(same_key, jgt).any(-1)      # [B,8]
    same_cell = cell[:, :, None] == cell[:, None, :]
    # mji[b,i,j] = mign[b, j, idxm[b, i]]
    mji = np.take_along_axis(
        mign.transpose(0, 2, 1),                        # [b, a, j]
        idxm[:, :, None], axis=1)                       # [b, i, j]
    ignov = np.logical_and(np.logical_and(same_cell, jgt), mji).any(-1)
    weff = np.where(ignov, np.float32(-1.0), weight)
    return dict(idxm=idxm, crf=crf, txy=txy, twh=twh, cell=cell,
                lastw=lastw.astype(np.float32), weff=weff.astype(np.float32),
                areag=areag, gbx=gbx)


def _make_meta(m: dict, sl: slice) -> np.ndarray:
    """Pack per-slot gather offsets + f32 consts for one core -> i32 [S,NMETA]."""
    idxm = m["idxm"][sl].reshape(S)
    cell = m["cell"][sl].reshape(S)
    crf = m["crf"][sl].reshape(S, 2)
    txy = m["txy"][sl].reshape(S, 2)
    twh = m["twh"][sl].reshape(S, 2)
    lastw = m["lastw"][sl].reshape(S)
    weff = m["weff"][sl].reshape(S)
    areag = m["areag"][sl].reshape(S)
    gbx = m["gbx"][sl].reshape(S, 4)
    lblch = m["lblch"][sl].reshape(S)

    img = np.arange(S) // NGT
    base = img * IMG + idxm * HW + cell                 # [S]
    # channel planes: conf=0, cls k -> 1+k, tx..th -> 81..84
    ch = np.concatenate([
        np.array([0, 81, 82, 81, 82, 83, 84, 83, 84]),
        np.zeros(1, np.int64),                          # placeholder for lbl
        np.arange(1, 81),
    ])
    offs = base[:, None] + ch[None, :] * (5 * HW)
    offs[:, 9] = base + lblch * (5 * HW)
    meta = np.zeros((S, NMETA), np.int32)
    meta[:, :NCOL] = offs.astype(np.int32)

    ct = np.zeros((S, NCONST), np.float32)
    anc = ANC[idxm]                                     # [S,2]
    ct[:, C_C4 + 0:C_C4 + 2] = crf
    ct[:, C_C4 + 2:C_C4 + 4] = -1.0 - crf
    ct[:, C_AH4 + 0:C_AH4 + 2] = anc * (GRID / 2.0)
    ct[:, C_AH4 + 2:C_AH4 + 4] = anc * (GRID / 2.0)
    ct[:, C_GB4 + 0] = gbx[:, 2] * GRID
    ct[:, C_GB4 + 1] = gbx[:, 3] * GRID
    ct[:, C_GB4 + 2] = -gbx[:, 0] * GRID
    ct[:, C_GB4 + 3] = -gbx[:, 1] * GRID
    ct[:, C_AGE] = areag * (HW * 1.0) + EPS * HW
    ct[:, C_LW] = lastw
    ct[:, C_LWE] = lastw * weff
    ct[:, C_ZC:C_ZC + 2] = 1.0 - txy
    ct[:, C_TWT:C_TWT + 2] = twh
    ct[:, C_ONE] = 1.0
    meta[:, NCOL:] = ct.view(np.int32)
    return meta


def build_bass() -> bass.Bass:
    nc = bass.Bass()
    py = nc.declare_dram_parameter("pyolos", [BL, 425, HW], F32, isOutput=False)
    mtd = nc.declare_dram_parameter("meta", [S, NMETA], I32, isOutput=False)
    out = nc.declare_dram_parameter("out", [1, 8], F32, isOutput=True)
    py_flat = py[:, :, :].rearrange("a b c -> (a b c)")

    with TileContext(nc) as tc:
        with (
            tc.tile_pool(name="sb", bufs=1) as sb,
            tc.tile_pool(name="ps", bufs=1, space="PSUM") as ps,
        ):
            mt = sb.tile([S, NMETA], I32, name="mt")
            conf = sb.tile([BL * 5, HW], F32, name="conf")
            sq = sb.tile([BL * 5, HW], F32, name="sq")
            pf = sb.tile([S, NCOL], F32, name="pf")
            u9 = sb.tile([S, 9], F32, name="u9")
            sp80 = sb.tile([S, 80], F32, name="sp80")
            stack = sb.tile([S, 8], F32, name="stack")

            def ctf(c0, c1):
                return mt[:, NCOL + c0:NCOL + c1].bitcast(F32)

            def tt(shape, tag):
                return sb.tile(shape, F32, name=tag)

            # ---- DMAs: meta on SP; conf halves split across both HWDGE
            # queues so pass 1 can start earlier ----
            CH = 338
            nc.sync.dma_start(out=mt[:], in_=mtd[:, :])
            nc.scalar.dma_start(out=conf[:, 0:CH], in_=py[:, 0:5, 0:CH])
            nc.sync.dma_start(out=conf[:, CH:HW], in_=py[:, 0:5, CH:HW])

            # ---- indirect gathers on gpsimd (critical 10 cols first) ----
            in1 = py_flat.rearrange("(a b) -> a b", b=1)
            nc.gpsimd.indirect_dma_start(
                out=pf[:, 0:10], out_offset=None, in_=in1,
                in_offset=bass.IndirectOffsetOnAxis(ap=mt[:, 0:10], axis=0))
            nc.gpsimd.indirect_dma_start(
                out=pf[:, 10:NCOL], out_offset=None, in_=in1,
                in_offset=bass.IndirectOffsetOnAxis(ap=mt[:, 10:NCOL], axis=0))

            # ---- DVE early (independent of gathers): zero the stack ----
            nc.vector.memset(stack[:], 0.0)

            # ---- ACT queue.  Dense conf background
            # sigmoid(x)^2 = exp(-2*ln(1+e^-x)) in 3 passes, column-split so
            # the per-slot activations interleave at pf/cls arrival. ----
            nc.scalar.activation(sq[:, 0:CH], conf[:, 0:CH], AF.Exp,
                                 scale=-1.0)
            nc.scalar.activation(sq[:, CH:HW], conf[:, CH:HW], AF.Exp,
                                 scale=-1.0)
            nc.scalar.activation(sq[:, 0:CH], sq[:, 0:CH], AF.Ln, bias=1.0)
            nc.scalar.activation(u9[:, 0:3], pf[:, 0:3], AF.Exp, scale=-1.0)
            nc.scalar.activation(u9[:, 3:9], pf[:, 3:9], AF.Exp)
            spl = tt([S, 2], "spl")
            nc.scalar.activation(spl[:], u9[:, 1:3], AF.Ln, bias=1.0)
            nc.scalar.activation(sq[:, CH:HW], sq[:, CH:HW], AF.Ln, bias=1.0)
            nc.scalar.activation(sq[:, 0:CH], sq[:, 0:CH], AF.Exp, scale=-2.0,
                                 accum_out=stack[0:BL * 5, 0:1])
            spsum = tt([S, 1], "spsum")
            nc.scalar.activation(sp80[:], pf[:, 10:NCOL], AF.Exp)
            nc.scalar.activation(sp80[:], sp80[:], AF.Ln, bias=1.0,
                                 accum_out=spsum[:])
            nc.scalar.activation(sq[:, CH:HW], sq[:, CH:HW], AF.Exp,
                                 scale=-2.0,
                                 accum_out=stack[0:BL * 5, 7:8])

            # ---- DVE pre-chain (fills the wait for u3/e6): bce-xy / mse-wh
            # elementwise pieces that only need the gathered columns ----
            t2 = tt([S, 2], "t2")
            nc.vector.tensor_tensor(out=t2[:], in0=pf[:, 1:3],
                                    in1=ctf(C_ZC, C_ZC + 2), op=OP.mult)
            dwh = tt([S, 2], "dwh")
            nc.vector.tensor_tensor(out=dwh[:], in0=pf[:, 5:7],
                                    in1=ctf(C_TWT, C_TWT + 2), op=OP.subtract)
            dsq = tt([S, 2], "dsq")
            nc.vector.tensor_tensor(out=dsq[:], in0=dwh[:], in1=dwh[:],
                                    op=OP.mult)
            msewh = tt([S, 1], "msewh")
            nc.vector.tensor_tensor(out=msewh[:], in0=dsq[:, 0:1],
                                    in1=dsq[:, 1:2], op=OP.add)

            # ---- Pool strand: pwh/area pieces feeding the DVE chain ----
            w4 = tt([S, 4], "w4")
            nc.gpsimd.tensor_tensor(out=w4[:], in0=u9[:, 5:9],
                                    in1=ctf(C_AH4, C_AH4 + 4), op=OP.mult)
            w4c = tt([S, 4], "w4c")
            nc.gpsimd.tensor_tensor(out=w4c[:], in0=w4[:],
                                    in1=ctf(C_C4, C_C4 + 4), op=OP.add)
            pa = tt([S, 1], "pa")
            nc.gpsimd.tensor_tensor(out=pa[:], in0=w4[:, 0:1], in1=w4[:, 1:2],
                                    op=OP.mult)
            pa_ag = tt([S, 1], "pa_ag")
            nc.gpsimd.tensor_scalar(pa_ag[:], pa[:], 4.0,
                                    ctf(C_AGE, C_AGE + 1), OP.mult, OP.add)

            # ---- DVE decode/IoU chain ----
            v5 = tt([S, 5], "v5")
            nc.vector.tensor_scalar(v5[:], u9[:, 0:5], 1.0, None, OP.add)
            sig5 = tt([S, 5], "sig5")
            nc.vector.reciprocal(sig5[:], v5[:])
            q4 = tt([S, 4], "q4")
            nc.vector.tensor_tensor(out=q4[:], in0=sig5[:, 1:5], in1=w4c[:],
                                    op=OP.add)
            q4m = tt([S, 4], "q4m")
            nc.vector.tensor_tensor(out=q4m[:], in0=q4[:],
                                    in1=ctf(C_GB4, C_GB4 + 4), op=OP.min)
            s2 = tt([S, 2], "s2")
            nc.vector.tensor_tensor(out=s2[:], in0=q4m[:, 0:2],
                                    in1=q4m[:, 2:4], op=OP.add)
            iwh = tt([S, 2], "iwh")
            nc.vector.tensor_scalar(iwh[:], s2[:], 0.0, None, OP.max)
            inter = tt([S, 1], "inter")
            nc.vector.tensor_tensor(out=inter[:], in0=iwh[:, 0:1],
                                    in1=iwh[:, 1:2], op=OP.mult)
            den = tt([S, 1], "den")
            nc.vector.scalar_tensor_tensor(out=den[:], in0=inter[:],
                                           scalar=-1.0, in1=pa_ag[:],
                                           op0=OP.mult, op1=OP.add)
            deni = tt([S, 1], "deni")
            nc.vector.reciprocal(deni[:], den[:])
            gconf = tt([S, 1], "gconf")
            nc.vector.tensor_tensor(out=gconf[:], in0=inter[:], in1=deni[:],
                                    op=OP.mult)
            # mp = (gconf > 0) * lastw, written straight into the npos col
            nc.vector.tensor_scalar(stack[:, 3:4], gconf[:], 0.0,
                                    ctf(C_LW, C_LW + 1), OP.is_gt, OP.mult)
            mpw = tt([S, 1], "mpw")
            nc.vector.tensor_scalar(mpw[:], gconf[:], 0.0,
                                    ctf(C_LWE, C_LWE + 1), OP.is_gt, OP.mult)
            dconf = tt([S, 1], "dconf")
            nc.vector.tensor_tensor(out=dconf[:], in0=sig5[:, 0:1],
                                    in1=gconf[:], op=OP.subtract)
            nc.vector.scalar_tensor_tensor(out=stack[:, 1:2], in0=dconf[:],
                                           scalar=stack[:, 3:4], in1=dconf[:],
                                           op0=OP.mult, op1=OP.mult)
            nc.vector.scalar_tensor_tensor(out=stack[:, 2:3], in0=sig5[:, 0:1],
                                           scalar=stack[:, 3:4],
                                           in1=sig5[:, 0:1],
                                           op0=OP.mult, op1=OP.mult)

            # ---- Pool strand: remaining stack columns ----
            nc.gpsimd.tensor_scalar(stack[:, 6:7], msewh[:], mpw[:], None,
                                    OP.mult)
            sptxy = tt([S, 2], "sptxy")
            nc.gpsimd.tensor_tensor(out=sptxy[:], in0=spl[:], in1=t2[:],
                                    op=OP.add)
            bcexy = tt([S, 1], "bcexy")
            nc.gpsimd.tensor_tensor(out=bcexy[:], in0=sptxy[:, 0:1],
                                    in1=sptxy[:, 1:2], op=OP.add)
            nc.gpsimd.tensor_scalar(stack[:, 5:6], bcexy[:], mpw[:], None,
                                    OP.mult)

            # ---- DVE tail: cls column via mp*spsum - mp*xlab ----
            xm = tt([S, 1], "xm")
            nc.vector.tensor_tensor(out=xm[:], in0=pf[:, 9:10],
                                    in1=stack[:, 3:4], op=OP.mult)
            nc.vector.scalar_tensor_tensor(out=stack[:, 4:5], in0=spsum[:],
                                           scalar=stack[:, 3:4], in1=xm[:],
                                           op0=OP.mult, op1=OP.subtract)

            # ---- final reduce over partitions + output ----
            red = ps.tile([1, 8], F32, name="red")
            nc.tensor.matmul(out=red[:], lhsT=ctf(C_ONE, C_ONE + 1),
                             rhs=stack[:], start=True, stop=True)
            osb = sb.tile([1, 8], F32, name="osb")
            nc.vector.tensor_copy(osb[:], red[:])
            nc.sync.dma_start(out=out[:, :], in_=osb[:])
    _split_multiwaits(nc, k=1)
    return nc


_NC_CACHE = None
LAST_RESULTS = None


def _get_nc():
    global _NC_CACHE
    if _NC_CACHE is None:
        _NC_CACHE = build_bass()
    return _NC_CACHE


def run(pyolos, gboxes_ltrb, labels, trace=False, **spmd_kwargs):
    global LAST_RESULTS
    nc = _get_nc()
    py = np.ascontiguousarray(
        np.asarray(pyolos, np.float32).reshape(B, 425, HW))
    gbx = np.asarray(gboxes_ltrb, np.float32).reshape(B, NGT, 4)
    lbl = np.asarray(labels).reshape(B, NGT).astype(np.int64)
    m = _host_match(gbx, lbl)
    m["lblch"] = lbl  # class channel plane index is exactly the 1-based label
    in_maps = []
    for c in range(NC):
        sl = slice(c * BL, (c + 1) * BL)
        in_maps.append({
            "pyolos": py[sl],
            "meta": _make_meta(m, sl),
        })
    res = run_bass_kernel_spmd(nc, in_maps, list(range(NC)), trace=trace,
                               **spmd_kwargs)
    LAST_RESULTS = res
    outs = np.stack([r["out"][0] for r in res.results]).astype(np.float64)
    t = outs.sum(0)
    dense_sq = t[0] + t[7]
    pos_mse, pos_psq, npos, cls_num, txy_s, twh_s = t[1:7]
    loss = (5.0 * pos_mse / B
            + (dense_sq - pos_psq) / B
            + cls_num / max(npos, 1.0)
            + txy_s / B
            + twh_s / B)
    return np.float32(loss)


def kernel(pyolos, gboxes_ltrb, labels):
    return run(pyolos, gboxes_ltrb, labels)


# revision 39
# speedup vs baseline: 1.0425x; 1.0334x over previous
"""YOLO-v2 loss kernel for Trainium2 (8 NeuronCores, data-parallel over batch).

Decomposition (same partial sums as the validated baseline):
  stack cols = [dense_sigmoid_sq_A, pos_mse, pos_psq, npos, cls_num, txy, twh,
                dense_sigmoid_sq_B]
summed over 128 partitions by one PE matmul, combined on the host.

The GT matching (anchors IoU, argmax, cell assignment, last-writer-wins and
ignore-overwrite masks) depends only on gboxes/labels (4KB of input), so it is
precomputed on the host into per-slot constants + gather offsets, exactly like
the baseline already precomputed its per-slot channel offsets.  Everything
touching pyolos (the 147MB tensor) stays on-device:
  - conf planes (16x5x676 per core) DMA'd + sigmoid^2-accumulate split between
    the ACT engine (3-pass exp/ln, cols 0:LD) and the DVE (recip path, rest)
  - 90 channel values per slot fetched with one indirect DMA
  - decode/IoU/loss math on DVE+Pool in parallel strands

Key trick: columns are gathered in the order
  [conf, tx, ty, tx, ty, tw, th, tw, th, lbl_ch, cls0..79]
with tx/ty duplicated so one Exp(-x) activation plus a DVE reciprocal yields
both sigmoid(t) and sigmoid(-t) = 1 - sigmoid(t); the prb / -plt box corners
then come out of a single add against host-folded constants, and the IoU
min/max pair collapses to one tensor_tensor min.
"""

import numpy as np

from concourse import bass, mybir
from concourse.bass_utils import run_bass_kernel_spmd
from concourse.tile import TileContext

F32 = mybir.dt.float32
I32 = mybir.dt.int32
AF = mybir.ActivationFunctionType
OP = mybir.AluOpType
AX = mybir.AxisListType

NC = 8                 # cores
B = 128                # batch
BL = B // NC           # images per core (16)
NGT = 8                # GTs per image
S = BL * NGT           # slots per core (128)
GRID = 26
HW = GRID * GRID       # 676
IMG = 425 * HW         # elements per image
EPS = 1e-7
NCOL = 90              # gathered columns per slot
NCONST = 24            # f32 const columns
NMETA = NCOL + NCONST  # i32 meta columns (consts bitcast)
LD = 376               # dense-conf columns handled by ACT (rest on DVE)
ANC = np.array([[0.05, 0.07], [0.12, 0.15], [0.25, 0.30],
                [0.45, 0.50], [0.80, 0.85]], np.float32)

# const f32 column offsets (within the 24-col block)
C_C4 = 0      # [4]  [cr0, cr1, -1-cr0, -1-cr1]
C_AH4 = 4     # [4]  [aw, ah, aw, ah] * 13
C_GB4 = 8     # [4]  [r, b, -l, -t] * 26
C_AGE = 12    # [1]  gt area*676 + eps*676
C_LW = 13     # [1]  last-writer mask
C_LWE = 14    # [1]  lw * weff
C_ZC = 15     # [2]  1 - txy target
C_TWT = 17    # [2]  twh target
C_ONE = 19    # [1]  1.0


def _split_multiwaits(nc: bass.Bass, k: int = 1) -> None:
    """This walrus build rejects instructions with >~2 sync waits; hoist
    extra waits onto preceding same-engine NoOps."""
    for fn in nc.m.functions:
        for bb in fn.blocks:
            out = []
            for inst in bb.instructions:
                si = inst.sync_info
                waits = list(si.on_wait) if si is not None and si.on_wait else []
                if len(waits) > k:
                    for i, w in enumerate(waits[:-k]):
                        out.append(mybir.InstNoOp(
                            name=f"{inst.name}-wsplit{i}",
                            engine=inst.engine,
                            bass_nofuse=True,
                            sync_info=mybir.SyncInfo(on_wait=[w],
                                                     on_update=[]),
                        ))
                    inst.sync_info = mybir.SyncInfo(
                        on_wait=waits[-k:], on_update=list(si.on_update))
                out.append(inst)
            bb.instructions = out


def _host_match(gbx: np.ndarray, lbl: np.ndarray):
    """Vectorized fmatch4yolov2 mirror (f32, matches the jax reference).
    gbx [B,8,4] ltrb, lbl [B,8] 1-based.  Returns per-slot meta arrays."""
    gbx = gbx.astype(np.float32)
    cxy = (gbx[..., :2] + gbx[..., 2:]) * np.float32(0.5)
    wh = gbx[..., 2:] - gbx[..., :2]
    inter = np.minimum(wh[..., None, :], ANC[None, None]).prod(-1)
    areag = wh.prod(-1)
    iou2 = inter / (areag[..., None] + (ANC[:, 0] * ANC[:, 1])[None, None]
                    - inter + np.float32(EPS))
    mign = iou2 > 0.5                                   # [B,8,5]
    idxm = iou2.argmax(-1)                              # [B,8]
    colrow = (cxy * np.float32(GRID)).astype(np.int32)  # trunc == floor here
    crf = colrow.astype(np.float32)
    txy = cxy * np.float32(GRID) - crf
    twh = np.log(wh / ANC[idxm])
    weight = np.float32(2.0) - areag
    cell = colrow[..., 1] * GRID + colrow[..., 0]       # [B,8] int
    key = cell * 5 + idxm

    # upper-triangular (j > i) collision masks
    jgt = np.triu(np.ones((NGT, NGT), bool), 1)[None]   # [1,i,j]
    same_key = key[:, :, None] == key[:, None, :]       # [b,i,j]
    lastw = ~np.logical_and(same_key, jgt).any(-1)      # [B,8]
    same_cell = cell[:, :, None] == cell[:, None, :]
    # mji[b,i,j] = mign[b, j, idxm[b, i]]
    mji = np.take_along_axis(
        mign.transpose(0, 2, 1),                        # [b, a, j]
        idxm[:, :, None], axis=1)                       # [b, i, j]
    ignov = np.logical_and(np.logical_and(same_cell, jgt), mji).any(-1)
    weff = np.where(ignov, np.float32(-1.0), weight)
    return dict(idxm=idxm, crf=crf, txy=txy, twh=twh, cell=cell,
                lastw=lastw.astype(np.float32), weff=weff.astype(np.float32),
                areag=areag, gbx=gbx)


def _make_meta(m: dict, sl: slice) -> np.ndarray:
    """Pack per-slot gather offsets + f32 consts for one core -> i32 [S,NMETA]."""
    idxm = m["idxm"][sl].reshape(S)
    cell = m["cell"][sl].reshape(S)
    crf = m["crf"][sl].reshape(S, 2)
    txy = m["txy"][sl].reshape(S, 2)
    twh = m["twh"][sl].reshape(S, 2)
    lastw = m["lastw"][sl].reshape(S)
    weff = m["weff"][sl].reshape(S)
    areag = m["areag"][sl].reshape(S)
    gbx = m["gbx"][sl].reshape(S, 4)
    lblch = m["lblch"][sl].reshape(S)

    img = np.arange(S) // NGT
    base = img * IMG + idxm * HW + cell                 # [S]
    # channel planes: conf=0, cls k -> 1+k, tx..th -> 81..84
    ch = np.concatenate([
        np.array([0, 81, 82, 81, 82, 83, 84, 83, 84]),
        np.zeros(1, np.int64),                          # placeholder for lbl
        np.arange(1, 81),
    ])
    offs = base[:, None] + ch[None, :] * (5 * HW)
    offs[:, 9] = base + lblch * (5 * HW)
    meta = np.zeros((S, NMETA), np.int32)
    meta[:, :NCOL] = offs.astype(np.int32)

    ct = np.zeros((S, NCONST), np.float32)
    anc = ANC[idxm]                                     # [S,2]
    ct[:, C_C4 + 0:C_C4 + 2] = crf
    ct[:, C_C4 + 2:C_C4 + 4] = -1.0 - crf
    ct[:, C_AH4 + 0:C_AH4 + 2] = anc * (GRID / 2.0)
    ct[:, C_AH4 + 2:C_AH4 + 4] = anc * (GRID / 2.0)
    ct[:, C_GB4 + 0] = gbx[:, 2] * GRID
    ct[:, C_GB4 + 1] = gbx[:, 3] * GRID
    ct[:, C_GB4 + 2] = -gbx[:, 0] * GRID
    ct[:, C_GB4 + 3] = -gbx[:, 1] * GRID
    ct[:, C_AGE] = areag * (HW * 1.0) + EPS * HW
    ct[:, C_LW] = lastw
    ct[:, C_LWE] = lastw * weff
    ct[:, C_ZC:C_ZC + 2] = 1.0 - txy
    ct[:, C_TWT:C_TWT + 2] = twh
    ct[:, C_ONE] = 1.0
    meta[:, NCOL:] = ct.view(np.int32)
    return meta


def build_bass() -> bass.Bass:
    nc = bass.Bass()
    py = nc.declare_dram_parameter("pyolos", [BL, 425, HW], F32, isOutput=False)
    mtd = nc.declare_dram_parameter("meta", [S, NMETA], I32, isOutput=False)
    out = nc.declare_dram_parameter("out", [1, 8], F32, isOutput=True)
    py_flat = py[:, :, :].rearrange("a b c -> (a b c)")

    with TileContext(nc) as tc:
        with (
            tc.tile_pool(name="sb", bufs=1) as sb,
            tc.tile_pool(name="ps", bufs=1, space="PSUM") as ps,
        ):
            mt = sb.tile([S, NMETA], I32, name="mt")
            conf = sb.tile([BL * 5, HW], F32, name="conf")
            sq = sb.tile([BL * 5, HW], F32, name="sq")
            pf = sb.tile([S, NCOL], F32, name="pf")
            u9 = sb.tile([S, 9], F32, name="u9")
            sp80 = sb.tile([S, 80], F32, name="sp80")
            stack = sb.tile([S, 8], F32, name="stack")

            def ctf(c0, c1):
                return mt[:, NCOL + c0:NCOL + c1].bitcast(F32)

            def tt(shape, tag):
                return sb.tile(shape, F32, name=tag)

            # ---- DMAs: meta on SP; conf halves split across both HWDGE
            # queues so pass 1 can start earlier ----
            CH = 338
            nc.sync.dma_start(out=mt[:], in_=mtd[:, :])
            nc.scalar.dma_start(out=conf[:, 0:CH], in_=py[:, 0:5, 0:CH])
            nc.sync.dma_start(out=conf[:, CH:HW], in_=py[:, 0:5, CH:HW])

            # ---- indirect gathers on gpsimd (critical 10 cols first) ----
            in1 = py_flat.rearrange("(a b) -> a b", b=1)
            nc.gpsimd.indirect_dma_start(
                out=pf[:, 0:10], out_offset=None, in_=in1,
                in_offset=bass.IndirectOffsetOnAxis(ap=mt[:, 0:10], axis=0))
            nc.gpsimd.indirect_dma_start(
                out=pf[:, 10:NCOL], out_offset=None, in_=in1,
                in_offset=bass.IndirectOffsetOnAxis(ap=mt[:, 10:NCOL], axis=0))

            # ---- DVE early (independent of gathers): zero the stack ----
            nc.vector.memset(stack[:], 0.0)

            # ---- ACT queue.  Dense conf background
            # sigmoid(x)^2 = exp(-2*ln(1+e^-x)) in 3 passes, column-split so
            # the per-slot activations interleave at pf/cls arrival. ----
            nc.scalar.activation(sq[:, 0:CH], conf[:, 0:CH], AF.Exp,
                                 scale=-1.0)
            nc.scalar.activation(sq[:, CH:HW], conf[:, CH:HW], AF.Exp,
                                 scale=-1.0)
            nc.scalar.activation(sq[:, 0:CH], sq[:, 0:CH], AF.Ln, bias=1.0)
            nc.scalar.activation(u9[:, 0:3], pf[:, 0:3], AF.Exp, scale=-1.0)
            nc.scalar.activation(u9[:, 3:9], pf[:, 3:9], AF.Exp)
            spl = tt([S, 2], "spl")
            nc.scalar.activation(spl[:], u9[:, 1:3], AF.Ln, bias=1.0)
            nc.scalar.activation(sq[:, CH:HW], sq[:, CH:HW], AF.Ln, bias=1.0)
            nc.scalar.activation(sq[:, 0:CH], sq[:, 0:CH], AF.Exp, scale=-2.0,
                                 accum_out=stack[0:BL * 5, 0:1])
            spsum = tt([S, 1], "spsum")
            nc.scalar.activation(sp80[:], pf[:, 10:NCOL], AF.Exp)
            nc.scalar.activation(sp80[:], sp80[:], AF.Ln, bias=1.0,
                                 accum_out=spsum[:])
            nc.scalar.activation(sq[:, CH:HW], sq[:, CH:HW], AF.Exp,
                                 scale=-2.0,
                                 accum_out=stack[0:BL * 5, 7:8])

            # ---- DVE pre-chain (fills the wait for u3/e6): bce-xy / mse-wh
            # elementwise pieces that only need the gathered columns ----
            t2 = tt([S, 2], "t2")
            nc.vector.tensor_tensor(out=t2[:], in0=pf[:, 1:3],
                                    in1=ctf(C_ZC, C_ZC + 2), op=OP.mult)
            dwh = tt([S, 2], "dwh")
            nc.vector.tensor_tensor(out=dwh[:], in0=pf[:, 5:7],
                                    in1=ctf(C_TWT, C_TWT + 2), op=OP.subtract)
            dsq = tt([S, 2], "dsq")
            nc.vector.tensor_tensor(out=dsq[:], in0=dwh[:], in1=dwh[:],
                                    op=OP.mult)
            msewh = tt([S, 1], "msewh")
            nc.vector.tensor_tensor(out=msewh[:], in0=dsq[:, 0:1],
                                    in1=dsq[:, 1:2], op=OP.add)

            # ---- Pool strand: pwh/area pieces feeding the DVE chain ----
            w4 = tt([S, 4], "w4")
            nc.gpsimd.tensor_tensor(out=w4[:], in0=u9[:, 5:9],
                                    in1=ctf(C_AH4, C_AH4 + 4), op=OP.mult)
            w4c = tt([S, 4], "w4c")
            nc.gpsimd.tensor_tensor(out=w4c[:], in0=w4[:],
                                    in1=ctf(C_C4, C_C4 + 4), op=OP.add)
            pa = tt([S, 1], "pa")
            nc.gpsimd.tensor_tensor(out=pa[:], in0=w4[:, 0:1], in1=w4[:, 1:2],
                                    op=OP.mult)
            pa_ag = tt([S, 1], "pa_ag")
            nc.gpsimd.tensor_scalar(pa_ag[:], pa[:], 4.0,
                                    ctf(C_AGE, C_AGE + 1), OP.mult, OP.add)

            # ---- DVE decode/IoU chain ----
            v5 = tt([S, 5], "v5")
            nc.vector.tensor_scalar(v5[:], u9[:, 0:5], 1.0, None, OP.add)
            sig5 = tt([S, 5], "sig5")
            nc.vector.reciprocal(sig5[:], v5[:])
            q4 = tt([S, 4], "q4")
            nc.vector.tensor_tensor(out=q4[:], in0=sig5[:, 1:5], in1=w4c[:],
                                    op=OP.add)
            q4m = tt([S, 4], "q4m")
            nc.vector.tensor_tensor(out=q4m[:], in0=q4[:],
                                    in1=ctf(C_GB4, C_GB4 + 4), op=OP.min)
            s2 = tt([S, 2], "s2")
            nc.vector.tensor_tensor(out=s2[:], in0=q4m[:, 0:2],
                                    in1=q4m[:, 2:4], op=OP.add)
            iwh = tt([S, 2], "iwh")
            nc.vector.tensor_scalar(iwh[:], s2[:], 0.0, None, OP.max)
            inter = tt([S, 1], "inter")
            nc.vector.tensor_tensor(out=inter[:], in0=iwh[:, 0:1],
                                    in1=iwh[:, 1:2], op=OP.mult)
            den = tt([S, 1], "den")
            nc.vector.scalar_tensor_tensor(out=den[:], in0=inter[:],
                                           scalar=-1.0, in1=pa_ag[:],
                                           op0=OP.mult, op1=OP.add)
            deni = tt([S, 1], "deni")
            nc.vector.reciprocal(deni[:], den[:])
            gconf = tt([S, 1], "gconf")
            nc.vector.tensor_tensor(out=gconf[:], in0=inter[:], in1=deni[:],
                                    op=OP.mult)
            # mp = (gconf > 0) * lastw, written straight into the npos col
            nc.vector.tensor_scalar(stack[:, 3:4], gconf[:], 0.0,
                                    ctf(C_LW, C_LW + 1), OP.is_gt, OP.mult)
            mpw = tt([S, 1], "mpw")
            nc.vector.tensor_scalar(mpw[:], gconf[:], 0.0,
                                    ctf(C_LWE, C_LWE + 1), OP.is_gt, OP.mult)
            dconf = tt([S, 1], "dconf")
            nc.vector.tensor_tensor(out=dconf[:], in0=sig5[:, 0:1],
                                    in1=gconf[:], op=OP.subtract)
            nc.vector.scalar_tensor_tensor(out=stack[:, 1:2], in0=dconf[:],
                                           scalar=stack[:, 3:4], in1=dconf[:],
                                           op0=OP.mult, op1=OP.mult)
            nc.vector.scalar_tensor_tensor(out=stack[:, 2:3], in0=sig5[:, 0:1],
                                           scalar=stack[:, 3:4],
                                           in1=sig5[:, 0:1],
                                           op0=OP.mult, op1=OP.mult)

            # ---- Pool strand: remaining stack columns ----
            nc.gpsimd.tensor_scalar(stack[:, 6:7], msewh[:], mpw[:], None,
                                    OP.mult)
            sptxy = tt([S, 2], "sptxy")
            nc.gpsimd.tensor_tensor(out=sptxy[:], in0=spl[:], in1=t2[:],
                                    op=OP.add)
            bcexy = tt([S, 1], "bcexy")
            nc.gpsimd.tensor_tensor(out=bcexy[:], in0=sptxy[:, 0:1],
                                    in1=sptxy[:, 1:2], op=OP.add)
            nc.gpsimd.tensor_scalar(stack[:, 5:6], bcexy[:], mpw[:], None,
                                    OP.mult)

            # ---- DVE tail: cls column via mp*spsum - mp*xlab ----
            xm = tt([S, 1], "xm")
            nc.vector.tensor_tensor(out=xm[:], in0=pf[:, 9:10],
                                    in1=stack[:, 3:4], op=OP.mult)
            nc.vector.scalar_tensor_tensor(out=stack[:, 4:5], in0=spsum[:],
                                           scalar=stack[:, 3:4], in1=xm[:],
                                           op0=OP.mult, op1=OP.subtract)

            # ---- final reduce over partitions + output ----
            red = ps.tile([1, 8], F32, name="red")
            nc.tensor.matmul(out=red[:], lhsT=ctf(C_ONE, C_ONE + 1),
                             rhs=stack[:], start=True, stop=True)
            osb = sb.tile([1, 8], F32, name="osb")
            nc.vector.tensor_copy(osb[:], red[:])
            nc.sync.dma_start(out=out[:, :], in_=osb[:])
    _split_multiwaits(nc, k=1)
    return nc


_NC_CACHE = None
LAST_RESULTS = None


def _get_nc():
    global _NC_CACHE
    if _NC_CACHE is None:
        _NC_CACHE = build_bass()
    return _NC_CACHE


def run(pyolos, gboxes_ltrb, labels, trace=False, **spmd_kwargs):
    global LAST_RESULTS
    nc = _get_nc()
    py = np.ascontiguousarray(
        np.asarray(pyolos, np.float32).reshape(B, 425, HW))
    gbx = np.asarray(gboxes_ltrb, np.float32).reshape(B, NGT, 4)
    lbl = np.asarray(labels).reshape(B, NGT).astype(np.int64)
    m = _host_match(gbx, lbl)
    m["lblch"] = lbl  # class channel plane index is exactly the 1-based label
    in_maps = []
    for c in range(NC):
        sl = slice(c * BL, (c + 1) * BL)
        in_maps.append({
            "pyolos": py[sl],
            "meta": _make_meta(m, sl),
        })
    res = run_bass_kernel_spmd(nc, in_maps, list(range(NC)), trace=trace,
                               **spmd_kwargs)
    LAST_RESULTS = res
    outs = np.stack([r["out"][0] for r in res.results]).astype(np.float64)
    t = outs.sum(0)
    dense_sq = t[0] + t[7]
    pos_mse, pos_psq, npos, cls_num, txy_s, twh_s = t[1:7]
    loss = (5.0 * pos_mse / B
            + (dense_sq - pos_psq) / B
            + cls_num / max(npos, 1.0)
            + txy_s / B
            + twh_s / B)
    return np.float32(loss)


def kernel(pyolos, gboxes_ltrb, labels):
    return run(pyolos, gboxes_ltrb, labels)
